# revision 14
# baseline (speedup 1.0000x reference)
"""Trainium2 Bass kernel for nn_CSCLoss: multi-scale bilinear point-sampling
cosine-consistency loss.

loss = 1 - mean_{pairs,(b,n)} <normalize(sample(feat_i, p_bn)), normalize(sample(feat_j, p_bn))>

Sharding: data-parallel over batch — 32 images -> 8 cores x 4 images; the
host sums the 8 per-core partial sums and applies the 1 - total/count
epilogue (the all-reduce of the sharding hint, done on 8 scalars).

Per-core dataflow (DMA- and gather-balanced, ~ridge):
 - Feature maps stream through SBUF as merged-channel-half tiles
   [128ch, 2*H*W]; L0 (64x64, 16 MiB) on the ACT HWDGE ring, L1+L2 on SP.
 - All per-point math runs point-per-partition ([128, k] tiles, boxes are
   DMA'd to a [128pt, 4] layout), so phase A is ~130 short DVE ops instead
   of serial single-lane [1, N] chains.
 - gpsimd.ap_gather extracts bilinear corners. Its cost is per-INDEX
   (~28 ns/idx + ~4 us/dispatch), so the index count is the knob:
    * L0: plain f32 gather, 4 corner idxs per point per ch-half, one
      256-idx dispatch per image (hides under L0's own DMA time).
    * L1/L2: tiles are re-packed into bf16 PAIR arrays (u32 word p holds
      bf16 pixels (p, p+1); even pairs are the bf16 cast itself, odd pairs
      are two strided u16 copies on the ACT engine). One u32 gather then
      yields BOTH x-corners -> 2 idxs per point per ch-half, one 512-idx
      dispatch per level.
 - Gather indices are computed directly as int16 [128pt, cols] values and
   shuffled into ap_gather's wrapped [16]-replicated layout by a tiny
   DRAM round-trip on the idle SP ring (read-back AP does the permutation
   and the x8 core replication via a 0-stride broadcast dim).
 - Corner lerp weights take the same route ([128pt, 12] -> DRAM ->
   [1, 1024] rows) and are broadcast to 128 partitions by a PE rank-1
   matmul; DVE applies them and reduces corner groups of 4 into sampled
   vectors V[c, u*64 + half*32 + n].
 - Channel reductions (squared norms, pairwise dots) are ones-vector
   matmuls accumulating [1, 256] in PSUM + a strided add folding the two
   channel halves; the cosine epilogue runs on partition 0 and emits one
   [1,1] partial per core.
"""

import sys
from contextlib import ExitStack

import numpy as np

if "/opt/trn_rl_repo" not in sys.path:
    sys.path.insert(0, "/opt/trn_rl_repo")

B, N, C = 32, 32, 256
LEVELS = [(64, 64), (32, 32), (16, 16)]  # (H, W)
N_CORES = 8
BL = B // N_CORES          # images per core
NPTS = BL * N              # 128 points per core
PAIRS = [(0, 1), (0, 2), (1, 2)]
EPS = 1e-12

_CACHE = {}


def _build_program():
    from concourse import bacc, bass, mybir, tile, library_config

    dt = mybir.dt
    AL = mybir.AluOpType

    nc = bacc.Bacc("TRN2", target_bir_lowering=False, debug=False)

    feats = [
        nc.dram_tensor(f"feat{i}", [BL, C, H, W], dt.float32, kind="ExternalInput")
        for i, (H, W) in enumerate(LEVELS)
    ]
    boxes = nc.dram_tensor("boxes", [BL, N, 4], dt.float32, kind="ExternalInput")
    out = nc.dram_tensor("out", [1, 1], dt.float32, kind="ExternalOutput")

    with tile.TileContext(nc) as tc, ExitStack() as ctx:
        pool = ctx.enter_context(tc.tile_pool(name="sbuf", bufs=1))
        pa = ctx.enter_context(tc.tile_pool(name="pa", bufs=1))
        pstream = ctx.enter_context(tc.tile_pool(name="stream", bufs=1))
        pwork = ctx.enter_context(tc.tile_pool(name="work", bufs=2))
        ppsum = ctx.enter_context(tc.tile_pool(name="psum", bufs=1, space="PSUM"))
        pdram = ctx.enter_context(tc.tile_pool(name="dram", bufs=1, space="DRAM"))

        nc.gpsimd.load_library(library_config.ap_gather)

        # ---- constants (DVE only — keep gpsimd free for gathers) ----
        ones1 = pool.tile([1, 128], dt.float32)
        nc.vector.memset(ones1[:], 1.0)
        ones = pool.tile([128, 1], dt.float32)
        nc.vector.memset(ones[:], 1.0)
        # per-partition u*2 (identity layout: p = pt = u*32 + n)
        u2f = pool.tile([128, 1], dt.float32)
        for u in range(BL):
            nc.vector.memset(u2f[u * N:(u + 1) * N, :], float(u * 2))

        # ---- boxes -> [128pt, 4] on the SP ring ----
        bxp = pool.tile([128, 4], dt.float32)
        nc.sync.dma_start(out=bxp[:], in_=boxes.rearrange("b n c -> (b n) c"))

        # ---- feature streaming DMAs ----
        fviews = [
            feats[li].rearrange("b (h c) hh ww -> c b h (hh ww)", h=2)
            for li in range(3)
        ]
        T_tiles = {}
        # SP ring: L2 imgs then L1 imgs (small levels first -> early gathers)
        for li, bufs in ((2, 2), (1, 2)):
            HW = LEVELS[li][0] * LEVELS[li][1]
            for u in range(BL):
                T = pstream.tile(
                    [128, 2 * HW], dt.float32, name=f"T{li}_{u}",
                    tag=f"T{li}", bufs=bufs,
                )
                nc.sync.dma_start(
                    out=T[:].rearrange("c (h q) -> c h q", h=2),
                    in_=fviews[li][:, u],
                )
                T_tiles[(li, u)] = T
        # ACT ring: all of L0
        HW0 = LEVELS[0][0] * LEVELS[0][1]
        for u in range(BL):
            T = pstream.tile(
                [128, 2 * HW0], dt.float32, name=f"T0_{u}", tag="T0", bufs=3,
            )
            nc.scalar.dma_start(
                out=T[:].rearrange("c (h q) -> c h q", h=2),
                in_=fviews[0][:, u],
            )
            T_tiles[(0, u)] = T

        # ---- phase A ([128pt, k] math) ----
        def axis_prep(coord, E, name):
            """pf = clip(c*(E-1), 0, E-1); e0 = clamp(floor(pf), 0, E-2);
            we = pf - e0.  floor via 16.16 fixed point. All [128, 1]."""
            pf = pa.tile([128, 1], dt.float32, name=f"pf{name}", tag=f"pf{name}")
            nc.vector.tensor_scalar(
                out=pf[:], in0=coord, scalar1=float(E - 1), scalar2=0.0,
                op0=AL.mult, op1=AL.max,
            )
            nc.vector.tensor_scalar_min(out=pf[:], in0=pf[:], scalar1=float(E - 1))
            pxs = pa.tile([128, 1], dt.float32, name=f"pxs{name}", tag="pxs")
            nc.vector.tensor_scalar(
                out=pxs[:], in0=pf[:], scalar1=65536.0, scalar2=None, op0=AL.mult,
            )
            ifx = pa.tile([128, 1], dt.int32, name=f"ifx{name}", tag="ifx")
            nc.vector.tensor_copy(out=ifx[:], in_=pxs[:])
            x0i = pa.tile([128, 1], dt.int32, name=f"x0i{name}", tag="x0i")
            nc.vector.tensor_scalar(
                out=x0i[:], in0=ifx[:], scalar1=16, scalar2=None,
                op0=AL.arith_shift_right,
            )
            e0 = pa.tile([128, 1], dt.float32, name=f"e0{name}", tag=f"e0{name}")
            nc.vector.tensor_copy(out=e0[:], in_=x0i[:])
            nc.vector.tensor_scalar_min(out=e0[:], in0=e0[:], scalar1=float(E - 2))
            we = pa.tile([128, 1], dt.float32, name=f"we{name}", tag=f"we{name}")
            nc.vector.tensor_tensor(out=we[:], in0=pf[:], in1=e0[:], op=AL.subtract)
            return e0, we

        sdram = {}       # li -> DRAM bounce tile for indices
        s16_tiles = {}   # li -> int16 [128, cols] idx values
        wefs = {}        # li -> (wx, wy)

        def prep_level(li):
            H, W = LEVELS[li]
            HW = H * W
            x0f, wx = axis_prep(bxp[:, 0:1], W, f"x{li}")
            y0f, wy = axis_prep(bxp[:, 1:2], H, f"y{li}")
            wefs[li] = (wx, wy)
            qf = pa.tile([128, 1], dt.float32, name=f"qf{li}", tag="qf")
            nc.vector.tensor_scalar(
                out=qf[:], in0=y0f[:], scalar1=float(W), scalar2=None, op0=AL.mult,
            )
            nc.vector.tensor_tensor(out=qf[:], in0=qf[:], in1=x0f[:], op=AL.add)
            if li == 0:
                # S [128, 8] cols (h, k): q + h*HW + dk
                S = pa.tile([128, 8], dt.float32, name="S0f", tag="S0f")
                DK = [0.0, 1.0, float(W), float(W + 1)]
                for h in range(2):
                    for k in range(4):
                        nc.vector.tensor_scalar(
                            out=S[:, h * 4 + k:h * 4 + k + 1], in0=qf[:],
                            scalar1=DK[k] + h * float(HW), scalar2=None, op0=AL.add,
                        )
                S16 = pa.tile([128, 8], dt.int16, name="S0i", tag="S0i")
                nc.vector.tensor_copy(out=S16[:], in_=S[:])
                sd = pdram.tile([1, 1024], dt.int16, name="sd0")
            else:
                HW2 = HW // 2
                qi = pa.tile([128, 1], dt.int32, name=f"qi{li}", tag="qi")
                nc.vector.tensor_copy(out=qi[:], in_=qf[:])
                pari = pa.tile([128, 1], dt.int32, name=f"pari{li}", tag="pari")
                nc.vector.tensor_scalar(
                    out=pari[:], in0=qi[:], scalar1=1, scalar2=None,
                    op0=AL.bitwise_and,
                )
                shi = pa.tile([128, 1], dt.int32, name=f"shi{li}", tag="shi")
                nc.vector.tensor_scalar(
                    out=shi[:], in0=qi[:], scalar1=1, scalar2=None,
                    op0=AL.arith_shift_right,
                )
                parf = pa.tile([128, 1], dt.float32, name=f"parf{li}", tag="parf")
                nc.vector.tensor_copy(out=parf[:], in_=pari[:])
                shf = pa.tile([128, 1], dt.float32, name=f"shf{li}", tag="shf")
                nc.vector.tensor_copy(out=shf[:], in_=shi[:])
                # slot = (q>>1) + par*HW2; base = slot + u*2*HW
                slotf = pa.tile([128, 1], dt.float32, name=f"slotf{li}", tag="slotf")
                nc.vector.tensor_scalar(
                    out=slotf[:], in0=parf[:], scalar1=float(HW2), scalar2=None,
                    op0=AL.mult,
                )
                nc.vector.tensor_tensor(
                    out=slotf[:], in0=slotf[:], in1=shf[:], op=AL.add,
                )
                basef = pa.tile([128, 1], dt.float32, name=f"basef{li}", tag="basef")
                nc.vector.tensor_scalar(
                    out=basef[:], in0=u2f[:], scalar1=float(HW), scalar2=None,
                    op0=AL.mult,
                )
                nc.vector.tensor_tensor(
                    out=basef[:], in0=basef[:], in1=slotf[:], op=AL.add,
                )
                # S [128, 4] cols (h, row): base + h*HW + row*(W//2)
                S = pa.tile([128, 4], dt.float32, name=f"Sf{li}", tag=f"Sf{li}")
                for h in range(2):
                    for row in range(2):
                        nc.vector.tensor_scalar(
                            out=S[:, h * 2 + row:h * 2 + row + 1], in0=basef[:],
                            scalar1=float(h * HW + row * (W // 2)), scalar2=None,
                            op0=AL.add,
                        )
                S16 = pa.tile([128, 4], dt.int16, name=f"Si{li}", tag=f"Si{li}")
                nc.vector.tensor_copy(out=S16[:], in_=S[:])
                sd = pdram.tile([1, 512], dt.int16, name=f"sd{li}")
            sdram[li] = sd
            s16_tiles[li] = S16

        # phase A order: L2 -> L1 -> L0 (critical path = L2's chain)
        prep_level(2)
        prep_level(1)
        prep_level(0)

        # ---- index wrap: S16 -> DRAM -> [1,N] flat -> DVE permutation ->
        # srow (wrapped stream order) -> DRAM -> widx (x8 via 0-stride) ----
        widxs = {}
        for li in (2, 1, 0):
            ncol = 8 if li == 0 else 4
            NIDX = 128 * ncol
            sdA = pdram.tile([1, NIDX], dt.int16, name=f"sdA{li}")
            nc.sync.dma_start(out=sdA[:], in_=s16_tiles[li][:])
            v16 = pa.tile([1, NIDX], dt.int16, name=f"v16_{li}", tag="v16")
            nc.sync.dma_start(out=v16[:], in_=sdA[:])
            srow = pa.tile([1, NIDX], dt.int16, name=f"srow{li}", tag="srow")
            if li == 0:
                # stream (r=(nl0,k), c=(u,h,nh0)); nl0 = n%4, nh0 = n//4
                # srow[nl0, k, u, h, nh0] = v16[(u*32 + nh0*4 + nl0)*8 + h*4+k]
                sv = srow[:].rearrange(
                    "o (nl k u h nh) -> o nl k u h nh", nl=4, k=4, u=BL, h=2,
                )
                vv = v16[:].rearrange(
                    "o (u nh nl h k) -> o u nh nl h k", u=BL, nh=8, nl=4, h=2,
                )
                for h in range(2):
                    for k in range(4):
                        nc.vector.tensor_copy(
                            out=sv[:, :, k, :, h, :],
                            in_=vv[:, :, :, :, h, k].transpose([0, 3, 1, 2]),
                        )
                widx = pool.tile([128, 64], dt.int16, name="widx0")
            else:
                # stream (r=(nl8,row), c=(u,h,nh8)); nl8 = n%8, nh8 = n//8
                # srow[nl8, row, u, h, nh8] = v16[(u*32+nh8*8+nl8)*4 + h*2+row]
                sv = srow[:].rearrange(
                    "o (nl row u h nh) -> o nl row u h nh", nl=8, row=2, u=BL,
                    h=2,
                )
                vv = v16[:].rearrange(
                    "o (u nh nl h row) -> o u nh nl h row", u=BL, nh=4, nl=8,
                    h=2,
                )
                for h in range(2):
                    for row in range(2):
                        nc.vector.tensor_copy(
                            out=sv[:, :, row, :, h, :],
                            in_=vv[:, :, :, :, h, row].transpose([0, 3, 1, 2]),
                        )
                widx = pool.tile([128, 32], dt.int16, name=f"widx{li}")
            sdB = pdram.tile([1, NIDX], dt.int16, name=f"sdB{li}")
            nc.sync.dma_start(out=sdB[:], in_=srow[:])
            nc.sync.dma_start(
                out=widx[:], in_=sdB[:].to_broadcast([8, NIDX]),
            )
            widxs[li] = widx

        # ---- corner weights: [128(pi), 4] -> DRAM -> [1,512] -> wrow ----
        wkts = {}
        for li in range(3):
            wx, wy = wefs[li]
            w1x = pa.tile([128, 1], dt.float32, name=f"w1x{li}", tag="w1x")
            nc.vector.tensor_scalar(
                out=w1x[:], in0=wx[:], scalar1=-1.0, scalar2=1.0,
                op0=AL.mult, op1=AL.add,
            )
            w1y = pa.tile([128, 1], dt.float32, name=f"w1y{li}", tag="w1y")
            nc.vector.tensor_scalar(
                out=w1y[:], in0=wy[:], scalar1=-1.0, scalar2=1.0,
                op0=AL.mult, op1=AL.add,
            )
            wkt = pa.tile([128, 4], dt.float32, name=f"wkt{li}", tag=f"wkt{li}")
            for k, (wyt, wxt) in enumerate(
                [(w1y, w1x), (w1y, wx), (wy, w1x), (wy, wx)]
            ):
                nc.vector.tensor_tensor(
                    out=wkt[:, k:k + 1], in0=wyt[:], in1=wxt[:], op=AL.mult,
                )
            wd = pdram.tile([1, 512], dt.float32, name=f"wd{li}")
            nc.sync.dma_start(out=wd[:], in_=wkt[:])
            wsb = pa.tile([1, 512], dt.float32, name=f"wsb{li}", tag=f"wsb{li}")
            nc.sync.dma_start(out=wsb[:], in_=wd[:])
            wkts[li] = wsb

        def build_wrow(li, name):
            """wrow [1, 1024] col (u, h, n, kk) from the p-major [1, 512]
            bounce (identity layout -> one strided DVE copy per h)."""
            wsb = wkts[li]
            wrow = pa.tile([1, 1024], dt.float32, name=name, tag="wrow", bufs=1)
            wv = wrow[:].rearrange("o (u h q) -> o u h q", u=BL, h=2)
            sv = wsb[:].rearrange("o (u q) -> o u q", u=BL)
            for h in range(2):
                nc.vector.tensor_copy(out=wv[:, :, h, :], in_=sv[:])
            return wrow

        def broadcast_weights(wrow, name, out_dt):
            wb = pool.tile([128, 1024], out_dt, name=f"wb{name}")
            for c0 in (0, 512):
                wb_ps = ppsum.tile(
                    [128, 512], dt.float32, name=f"wbps{name}_{c0}", tag="wbps",
                    bufs=2,
                )
                nc.tensor.matmul(
                    wb_ps[:], ones1[:], wrow[:, c0:c0 + 512], start=True, stop=True,
                )
                nc.vector.tensor_copy(out=wb[:, c0:c0 + 512], in_=wb_ps[:])
            return wb

        wbs = {}
        for li in (2, 1, 0):
            wrow = build_wrow(li, f"wrow{li}")
            wbs[li] = broadcast_weights(
                wrow, f"L{li}", dt.bfloat16 if li else dt.float32,
            )

        # ---- bf16 pair packing for L1/L2 (DVE casts + ACT odd-pair copies) --
        packed = {}
        for li in (2, 1):
            HW = LEVELS[li][0] * LEVELS[li][1]
            HW2 = HW // 2
            P32 = pool.tile([128, 8 * HW], dt.int32, name=f"P32_{li}")
            Pb = P32[:].bitcast(dt.bfloat16)  # [128, 16*HW]
            for u in range(BL):
                T = T_tiles[(li, u)]
                for h in range(2):
                    base = (u * 2 + h) * 2 * HW
                    nc.vector.tensor_copy(
                        out=Pb[:, base:base + HW],
                        in_=T[:, h * HW:(h + 1) * HW],
                    )
                    bview = Pb[:, base:base + HW].rearrange(
                        "c (p two) -> c p two", two=2,
                    )
                    oview = Pb[:, base + HW:base + 2 * HW].rearrange(
                        "c (p two) -> c p two", two=2,
                    )
                    nc.scalar.copy(out=oview[:, :, 0], in_=bview[:, :, 1])
                    nc.scalar.copy(
                        out=oview[:, 0:HW2 - 1, 1], in_=bview[:, 1:HW2, 0],
                    )
            packed[li] = P32

        # ---- V tiles: col = u*64 + h*32 + n ----
        V = [pool.tile([128, 256], dt.float32, name=f"V{li}") for li in range(3)]

        def colsum(prod, name):
            ps = ppsum.tile([1, 256], dt.float32, name=f"ps{name}", tag="ps", bufs=2)
            nc.tensor.matmul(ps[:], ones[:], prod[:], start=True, stop=True)
            sb = pool.tile([1, 256], dt.float32, name=f"sb{name}")
            nc.vector.tensor_copy(out=sb[:], in_=ps[:])
            sbv = sb[:].rearrange("o (u h n) -> o u h n", u=BL, h=2)
            r = pool.tile([1, 128], dt.float32, name=f"r{name}")
            rv = r[:].rearrange("o (u n) -> o u n", u=BL)
            nc.vector.tensor_tensor(
                out=rv[:], in0=sbv[:, :, 0, :], in1=sbv[:, :, 1, :], op=AL.add,
            )
            return r

        results = {}
        done = set()

        def level_products(li):
            done.add(li)
            prod = pwork.tile([128, 256], dt.float32, name=f"pss{li}", tag="pc")
            nc.vector.tensor_tensor(
                out=prod[:], in0=V[li][:], in1=V[li][:], op=AL.mult,
            )
            results[f"ss{li}"] = colsum(prod, f"ss{li}")
            for (i, j) in PAIRS:
                if li in (i, j) and i in done and j in done:
                    prod = pwork.tile(
                        [128, 256], dt.float32, name=f"pd{i}{j}", tag="pc",
                    )
                    nc.vector.tensor_tensor(
                        out=prod[:], in0=V[i][:], in1=V[j][:], op=AL.mult,
                    )
                    results[f"d{i}{j}"] = colsum(prod, f"d{i}{j}")

        def gather_packed(li):
            HW = LEVELS[li][0] * LEVELS[li][1]
            og = pwork.tile([128, 512], dt.int32, name=f"ogp{li}", tag="ogp", bufs=1)
            nc.gpsimd.ap_gather(
                out_ap=og[:], in_ap=packed[li][:], idxs_ap=widxs[li][:],
                channels=128, num_elems=8 * HW, d=1, num_idxs=512,
            )
            prod = pwork.tile([128, 1024], dt.float32, name=f"lp{li}", tag="lp", bufs=1)
            nc.vector.tensor_tensor(
                out=prod[:], in0=og[:].bitcast(dt.bfloat16), in1=wbs[li][:],
                op=AL.mult,
            )
            nc.vector.tensor_reduce(
                out=V[li][:],
                in_=prod[:].rearrange("c (n f) -> c n f", f=4),
                axis=mybir.AxisListType.X, op=AL.add,
            )

        def gather_l0_img(u):
            og = pwork.tile([128, 256], dt.float32, name=f"og0_{u}", tag="og")
            nc.gpsimd.ap_gather(
                out_ap=og[:], in_ap=T_tiles[(0, u)][:],
                idxs_ap=widxs[0][:, u * 16:(u + 1) * 16],
                channels=128, num_elems=2 * HW0, d=1, num_idxs=256,
            )
            nc.vector.tensor_tensor(
                out=og[:], in0=og[:], in1=wbs[0][:, u * 256:(u + 1) * 256],
                op=AL.mult,
            )
            nc.vector.tensor_reduce(
                out=V[0][:, u * 64:(u + 1) * 64],
                in_=og[:].rearrange("c (n f) -> c n f", f=4),
                axis=mybir.AxisListType.X, op=AL.add,
            )

        gather_packed(2)
        level_products(2)
        gather_packed(1)
        level_products(1)
        for u in range(BL):
            gather_l0_img(u)
        level_products(0)

        # ---- cosine epilogue on partition 0 ----
        rns = []
        for li in range(3):
            nrm = pool.tile([1, 128], dt.float32, name=f"nrm{li}")
            nc.scalar.sqrt(out=nrm[:], in_=results[f"ss{li}"][:])
            nc.vector.tensor_scalar_max(out=nrm[:], in0=nrm[:], scalar1=EPS)
            rn = pool.tile([1, 128], dt.float32, name=f"rn{li}")
            nc.vector.reciprocal(out=rn[:], in_=nrm[:])
            rns.append(rn)

        tot = pool.tile([1, 128], dt.float32)
        first = True
        for i, j in PAIRS:
            t = pool.tile([1, 128], dt.float32, name=f"t{i}{j}")
            nc.vector.tensor_tensor(
                out=t[:], in0=results[f"d{i}{j}"][:], in1=rns[i][:], op=AL.mult,
            )
            nc.vector.tensor_tensor(out=t[:], in0=t[:], in1=rns[j][:], op=AL.mult)
            if first:
                nc.vector.tensor_copy(out=tot[:], in_=t[:])
                first = False
            else:
                nc.vector.tensor_tensor(out=tot[:], in0=tot[:], in1=t[:], op=AL.add)

        res = pool.tile([1, 1], dt.float32)
        nc.vector.tensor_reduce(
            out=res[:], in_=tot[:], axis=mybir.AxisListType.X, op=AL.add
        )
        nc.sync.dma_start(out=out.ap(), in_=res[:])

    nc.compile()
    return nc


def _get_program():
    if "nc" not in _CACHE:
        _CACHE["nc"] = _build_program()
    return _CACHE["nc"]


def _run_device(feat0, feat1, feat2, boxes, **run_kwargs):
    """Shard inputs batch-wise over the 8 cores, run the SPMD program, and
    return the BassKernelResults (one {"out": [1,1]} per core)."""
    from concourse.bass_utils import run_bass_kernel_spmd

    nc = _get_program()

    feats = [
        np.ascontiguousarray(np.asarray(f, dtype=np.float32))
        for f in (feat0, feat1, feat2)
    ]
    boxes = np.ascontiguousarray(np.asarray(boxes, dtype=np.float32))

    in_maps = []
    for k in range(N_CORES):
        sl = slice(k * BL, (k + 1) * BL)
        in_maps.append(
            {
                "feat0": feats[0][sl],
                "feat1": feats[1][sl],
                "feat2": feats[2][sl],
                "boxes": boxes[sl],
            }
        )

    return run_bass_kernel_spmd(
        nc, in_maps, core_ids=list(range(N_CORES)), **run_kwargs
    )


def kernel(feat0, feat1, feat2, boxes):
    r = _run_device(feat0, feat1, feat2, boxes)
    total = np.float64(0.0)
    for m in r.results:
        total += np.float64(m["out"].reshape(-1)[0])

    count = B * N * len(PAIRS)
    avg = np.float32(total) / np.float32(count)
    loss = np.float32(1.0) - avg
    loss = np.nan_to_num(loss, nan=0.0, posinf=1.0, neginf=0.0)
    return np.array(np.clip(loss, 0.0, 2.0), dtype=np.float32)


# revision 16
# speedup vs baseline: 1.1694x; 1.1694x over previous
"""Trainium2 Bass kernel for nn_CSCLoss: multi-scale bilinear point-sampling
cosine-consistency loss.

loss = 1 - mean_{pairs,(b,n)} <normalize(sample(feat_i, p_bn)), normalize(sample(feat_j, p_bn))>

Sharding: data-parallel over batch — 32 images -> 8 cores x 4 images; the
host sums the 8 per-core partial sums and applies the 1 - total/count
epilogue (the all-reduce of the sharding hint, done on 8 scalars).

Per-core dataflow (DMA- and gather-balanced, ~ridge):
 - Feature maps stream through SBUF as merged-channel-half tiles
   [128ch, 2*H*W]; L0 (64x64, 16 MiB) on the ACT HWDGE ring, L1+L2 on SP.
 - All per-point math runs point-per-partition ([128, k] tiles, boxes are
   DMA'd to a [128pt, 4] layout), so phase A is ~130 short DVE ops instead
   of serial single-lane [1, N] chains.
 - gpsimd.ap_gather extracts bilinear corners. Its cost is per-INDEX
   (~28 ns/idx + ~4 us/dispatch), so the index count is the knob:
    * L0: plain f32 gather, 4 corner idxs per point per ch-half, one
      256-idx dispatch per image (hides under L0's own DMA time).
    * L1/L2: tiles are re-packed into bf16 PAIR arrays (u32 word p holds
      bf16 pixels (p, p+1); even pairs are the bf16 cast itself, odd pairs
      are two strided u16 copies on the ACT engine). One u32 gather then
      yields BOTH x-corners -> 2 idxs per point per ch-half, one 512-idx
      dispatch per level.
 - Gather indices are computed directly as int16 [128pt, cols] values and
   shuffled into ap_gather's wrapped [16]-replicated layout by a tiny
   DRAM round-trip on the idle SP ring (read-back AP does the permutation
   and the x8 core replication via a 0-stride broadcast dim).
 - Corner lerp weights take the same route ([128pt, 12] -> DRAM ->
   [1, 1024] rows) and are broadcast to 128 partitions by a PE rank-1
   matmul; DVE applies them and reduces corner groups of 4 into sampled
   vectors V[c, u*64 + half*32 + n].
 - Channel reductions (squared norms, pairwise dots) are ones-vector
   matmuls accumulating [1, 256] in PSUM + a strided add folding the two
   channel halves; the cosine epilogue runs on partition 0 and emits one
   [1,1] partial per core.
"""

import sys
from contextlib import ExitStack

import numpy as np

if "/opt/trn_rl_repo" not in sys.path:
    sys.path.insert(0, "/opt/trn_rl_repo")

B, N, C = 32, 32, 256
LEVELS = [(64, 64), (32, 32), (16, 16)]  # (H, W)
N_CORES = 8
BL = B // N_CORES          # images per core
NPTS = BL * N              # 128 points per core
PAIRS = [(0, 1), (0, 2), (1, 2)]
EPS = 1e-12

_CACHE = {}


def _build_program():
    from concourse import bacc, bass, mybir, tile, library_config

    dt = mybir.dt
    AL = mybir.AluOpType

    nc = bacc.Bacc("TRN2", target_bir_lowering=False, debug=False)

    feats = [
        nc.dram_tensor(f"feat{i}", [BL, C, H, W], dt.float32, kind="ExternalInput")
        for i, (H, W) in enumerate(LEVELS)
    ]
    boxes = nc.dram_tensor("boxes", [BL, N, 4], dt.float32, kind="ExternalInput")
    out = nc.dram_tensor("out", [1, 1], dt.float32, kind="ExternalOutput")

    with tile.TileContext(nc) as tc, ExitStack() as ctx:
        pool = ctx.enter_context(tc.tile_pool(name="sbuf", bufs=1))
        pa = ctx.enter_context(tc.tile_pool(name="pa", bufs=1))
        pstream = ctx.enter_context(tc.tile_pool(name="stream", bufs=1))
        pwork = ctx.enter_context(tc.tile_pool(name="work", bufs=2))
        ppsum = ctx.enter_context(tc.tile_pool(name="psum", bufs=1, space="PSUM"))
        pdram = ctx.enter_context(tc.tile_pool(name="dram", bufs=1, space="DRAM"))

        nc.gpsimd.load_library(library_config.ap_gather)

        # ---- constants (DVE only — keep gpsimd free for gathers) ----
        ones1 = pool.tile([1, 128], dt.float32)
        nc.vector.memset(ones1[:], 1.0)
        ones = pool.tile([128, 1], dt.float32)
        nc.vector.memset(ones[:], 1.0)
        # per-partition u*2 (identity layout: p = pt = u*32 + n)
        u2f = pool.tile([128, 1], dt.float32)
        for u in range(BL):
            nc.vector.memset(u2f[u * N:(u + 1) * N, :], float(u * 2))

        # ---- boxes -> [128pt, 4] on the SP ring ----
        bxp = pool.tile([128, 4], dt.float32)
        nc.sync.dma_start(out=bxp[:], in_=boxes.rearrange("b n c -> (b n) c"))

        # ---- feature streaming DMAs ----
        # Queue plan (all queues are in-order; nothing early may wait on
        # anything late):  SP: boxes, T2 x4 (bufs=4, no reuse waits), idx
        # bounces, weight bounces, out.  ACT: T0 u0/u1, T1 x4 (bufs=4),
        # T0 u2, odd-pair copies, T0 u3 (its buffer-reuse wait on the first
        # L0 gather must not block the odd copies).
        fviews = [
            feats[li].rearrange("b (h c) hh ww -> c b h (hh ww)", h=2)
            for li in range(3)
        ]
        T_tiles = {}

        def stream_tile(li, u, bufs, eng):
            HW = LEVELS[li][0] * LEVELS[li][1]
            T = pstream.tile(
                [128, 2 * HW], dt.float32, name=f"T{li}_{u}",
                tag=f"T{li}", bufs=bufs,
            )
            eng.dma_start(
                out=T[:].rearrange("c (h q) -> c h q", h=2),
                in_=fviews[li][:, u],
            )
            T_tiles[(li, u)] = T

        HW0 = LEVELS[0][0] * LEVELS[0][1]
        for u in range(BL):
            stream_tile(2, u, 4, nc.sync)
        stream_tile(0, 0, 2, nc.scalar)
        stream_tile(0, 1, 2, nc.scalar)
        for u in range(BL):
            stream_tile(1, u, 4, nc.scalar)
        stream_tile(0, 2, 2, nc.scalar)

        # ---- phase A ([128pt, k] math) ----
        def axis_prep(coord, E, name):
            """pf = clip(c*(E-1), 0, E-1); e0 = clamp(floor(pf), 0, E-2);
            we = pf - e0.  floor via 16.16 fixed point. All [128, 1]."""
            pf = pa.tile([128, 1], dt.float32, name=f"pf{name}", tag=f"pf{name}")
            nc.vector.tensor_scalar(
                out=pf[:], in0=coord, scalar1=float(E - 1), scalar2=0.0,
                op0=AL.mult, op1=AL.max,
            )
            nc.vector.tensor_scalar_min(out=pf[:], in0=pf[:], scalar1=float(E - 1))
            pxs = pa.tile([128, 1], dt.float32, name=f"pxs{name}", tag="pxs")
            nc.vector.tensor_scalar(
                out=pxs[:], in0=pf[:], scalar1=65536.0, scalar2=None, op0=AL.mult,
            )
            ifx = pa.tile([128, 1], dt.int32, name=f"ifx{name}", tag="ifx")
            nc.vector.tensor_copy(out=ifx[:], in_=pxs[:])
            x0i = pa.tile([128, 1], dt.int32, name=f"x0i{name}", tag="x0i")
            nc.vector.tensor_scalar(
                out=x0i[:], in0=ifx[:], scalar1=16, scalar2=None,
                op0=AL.arith_shift_right,
            )
            e0 = pa.tile([128, 1], dt.float32, name=f"e0{name}", tag=f"e0{name}")
            nc.vector.tensor_copy(out=e0[:], in_=x0i[:])
            nc.vector.tensor_scalar_min(out=e0[:], in0=e0[:], scalar1=float(E - 2))
            we = pa.tile([128, 1], dt.float32, name=f"we{name}", tag=f"we{name}")
            nc.vector.tensor_tensor(out=we[:], in0=pf[:], in1=e0[:], op=AL.subtract)
            return e0, we

        sdram = {}       # li -> DRAM bounce tile for indices
        s16_tiles = {}   # li -> int16 [128, cols] idx values
        wefs = {}        # li -> (wx, wy)

        def prep_level(li):
            H, W = LEVELS[li]
            HW = H * W
            x0f, wx = axis_prep(bxp[:, 0:1], W, f"x{li}")
            y0f, wy = axis_prep(bxp[:, 1:2], H, f"y{li}")
            wefs[li] = (wx, wy)
            qf = pa.tile([128, 1], dt.float32, name=f"qf{li}", tag="qf")
            nc.vector.tensor_scalar(
                out=qf[:], in0=y0f[:], scalar1=float(W), scalar2=None, op0=AL.mult,
            )
            nc.vector.tensor_tensor(out=qf[:], in0=qf[:], in1=x0f[:], op=AL.add)
            if li == 0:
                # S [128, 8] cols (h, k): q + h*HW + dk
                S = pa.tile([128, 8], dt.float32, name="S0f", tag="S0f")
                DK = [0.0, 1.0, float(W), float(W + 1)]
                for h in range(2):
                    for k in range(4):
                        nc.vector.tensor_scalar(
                            out=S[:, h * 4 + k:h * 4 + k + 1], in0=qf[:],
                            scalar1=DK[k] + h * float(HW), scalar2=None, op0=AL.add,
                        )
                S16 = pa.tile([128, 8], dt.int16, name="S0i", tag="S0i")
                nc.vector.tensor_copy(out=S16[:], in_=S[:])
                sd = pdram.tile([1, 1024], dt.int16, name="sd0")
            else:
                HW2 = HW // 2
                qi = pa.tile([128, 1], dt.int32, name=f"qi{li}", tag="qi")
                nc.vector.tensor_copy(out=qi[:], in_=qf[:])
                pari = pa.tile([128, 1], dt.int32, name=f"pari{li}", tag="pari")
                nc.vector.tensor_scalar(
                    out=pari[:], in0=qi[:], scalar1=1, scalar2=None,
                    op0=AL.bitwise_and,
                )
                shi = pa.tile([128, 1], dt.int32, name=f"shi{li}", tag="shi")
                nc.vector.tensor_scalar(
                    out=shi[:], in0=qi[:], scalar1=1, scalar2=None,
                    op0=AL.arith_shift_right,
                )
                parf = pa.tile([128, 1], dt.float32, name=f"parf{li}", tag="parf")
                nc.vector.tensor_copy(out=parf[:], in_=pari[:])
                shf = pa.tile([128, 1], dt.float32, name=f"shf{li}", tag="shf")
                nc.vector.tensor_copy(out=shf[:], in_=shi[:])
                # slot = (q>>1) + par*HW2; base = slot + u*2*HW
                slotf = pa.tile([128, 1], dt.float32, name=f"slotf{li}", tag="slotf")
                nc.vector.tensor_scalar(
                    out=slotf[:], in0=parf[:], scalar1=float(HW2), scalar2=None,
                    op0=AL.mult,
                )
                nc.vector.tensor_tensor(
                    out=slotf[:], in0=slotf[:], in1=shf[:], op=AL.add,
                )
                basef = pa.tile([128, 1], dt.float32, name=f"basef{li}", tag="basef")
                nc.vector.tensor_scalar(
                    out=basef[:], in0=u2f[:], scalar1=float(HW), scalar2=None,
                    op0=AL.mult,
                )
                nc.vector.tensor_tensor(
                    out=basef[:], in0=basef[:], in1=slotf[:], op=AL.add,
                )
                # S [128, 4] cols (h, row): base + h*HW + row*(W//2)
                S = pa.tile([128, 4], dt.float32, name=f"Sf{li}", tag=f"Sf{li}")
                for h in range(2):
                    for row in range(2):
                        nc.vector.tensor_scalar(
                            out=S[:, h * 2 + row:h * 2 + row + 1], in0=basef[:],
                            scalar1=float(h * HW + row * (W // 2)), scalar2=None,
                            op0=AL.add,
                        )
                S16 = pa.tile([128, 4], dt.int16, name=f"Si{li}", tag=f"Si{li}")
                nc.vector.tensor_copy(out=S16[:], in_=S[:])
                sd = pdram.tile([1, 512], dt.int16, name=f"sd{li}")
            sdram[li] = sd
            s16_tiles[li] = S16

        widxs = {}

        def wrap_idx(li):
            """S16 -> DRAM -> [1,N] flat -> DVE permutation -> srow (wrapped
            stream order) -> DRAM -> widx (x8 replication via 0-stride)."""
            ncol = 8 if li == 0 else 4
            NIDX = 128 * ncol
            sdA = pdram.tile([1, NIDX], dt.int16, name=f"sdA{li}")
            nc.sync.dma_start(out=sdA[:], in_=s16_tiles[li][:])
            v16 = pa.tile([1, NIDX], dt.int16, name=f"v16_{li}", tag=f"v16_{li}")
            nc.sync.dma_start(out=v16[:], in_=sdA[:])
            srow = pa.tile([1, NIDX], dt.int16, name=f"srow{li}", tag=f"srow{li}")
            if li == 0:
                # stream (r=(nl0,k), c=(u,h,nh0)); nl0 = n%4, nh0 = n//4
                sv = srow[:].rearrange(
                    "o (nl k u h nh) -> o nl k u h nh", nl=4, k=4, u=BL, h=2,
                )
                vv = v16[:].rearrange(
                    "o (u nh nl h k) -> o u nh nl h k", u=BL, nh=8, nl=4, h=2,
                )
                for h in range(2):
                    for k in range(4):
                        nc.vector.tensor_copy(
                            out=sv[:, :, k, :, h, :],
                            in_=vv[:, :, :, :, h, k].transpose([0, 3, 1, 2]),
                        )
                widx = pool.tile([128, 64], dt.int16, name="widx0")
            else:
                # stream (r=(nl8,row), c=(u,h,nh8)); nl8 = n%8, nh8 = n//8
                sv = srow[:].rearrange(
                    "o (nl row u h nh) -> o nl row u h nh", nl=8, row=2, u=BL,
                    h=2,
                )
                vv = v16[:].rearrange(
                    "o (u nh nl h row) -> o u nh nl h row", u=BL, nh=4, nl=8,
                    h=2,
                )
                for h in range(2):
                    for row in range(2):
                        nc.vector.tensor_copy(
                            out=sv[:, :, row, :, h, :],
                            in_=vv[:, :, :, :, h, row].transpose([0, 3, 1, 2]),
                        )
                widx = pool.tile([128, 32], dt.int16, name=f"widx{li}")
            sdB = pdram.tile([1, NIDX], dt.int16, name=f"sdB{li}")
            nc.sync.dma_start(out=sdB[:], in_=srow[:])
            nc.sync.dma_start(
                out=widx[:], in_=sdB[:].to_broadcast([8, NIDX]),
            )
            widxs[li] = widx

        # ---- bf16 pair packing for L1/L2 (DVE casts + ACT odd copies) ----
        packed = {}

        def pack_level(li):
            HW = LEVELS[li][0] * LEVELS[li][1]
            HW2 = HW // 2
            P32 = pool.tile([128, 8 * HW], dt.int32, name=f"P32_{li}")
            Pb = P32[:].bitcast(dt.bfloat16)  # [128, 16*HW]
            for u in range(BL):
                T = T_tiles[(li, u)]
                for h in range(2):
                    base = (u * 2 + h) * 2 * HW
                    nc.vector.tensor_copy(
                        out=Pb[:, base:base + HW],
                        in_=T[:, h * HW:(h + 1) * HW],
                    )
                    bview = Pb[:, base:base + HW].rearrange(
                        "c (p two) -> c p two", two=2,
                    )
                    oview = Pb[:, base + HW:base + 2 * HW].rearrange(
                        "c (p two) -> c p two", two=2,
                    )
                    nc.scalar.copy(out=oview[:, :, 0], in_=bview[:, :, 1])
                    nc.scalar.copy(
                        out=oview[:, 0:HW2 - 1, 1], in_=bview[:, 1:HW2, 0],
                    )
            packed[li] = P32

        # per level: phase A -> index wrap -> packing (L2 first: its gather
        # can start the moment the gpsimd library load finishes)
        prep_level(2)
        wrap_idx(2)
        pack_level(2)
        prep_level(1)
        wrap_idx(1)
        pack_level(1)
        prep_level(0)
        wrap_idx(0)

        # ---- corner weights: [128(pt), 4] -> DRAM -> [1,512] -> wrow ----
        wkts = {}
        for li in range(3):
            wx, wy = wefs[li]
            w1x = pa.tile([128, 1], dt.float32, name=f"w1x{li}", tag="w1x")
            nc.vector.tensor_scalar(
                out=w1x[:], in0=wx[:], scalar1=-1.0, scalar2=1.0,
                op0=AL.mult, op1=AL.add,
            )
            w1y = pa.tile([128, 1], dt.float32, name=f"w1y{li}", tag="w1y")
            nc.vector.tensor_scalar(
                out=w1y[:], in0=wy[:], scalar1=-1.0, scalar2=1.0,
                op0=AL.mult, op1=AL.add,
            )
            wkt = pa.tile([128, 4], dt.float32, name=f"wkt{li}", tag=f"wkt{li}")
            for k, (wyt, wxt) in enumerate(
                [(w1y, w1x), (w1y, wx), (wy, w1x), (wy, wx)]
            ):
                nc.vector.tensor_tensor(
                    out=wkt[:, k:k + 1], in0=wyt[:], in1=wxt[:], op=AL.mult,
                )
            wd = pdram.tile([1, 512], dt.float32, name=f"wd{li}")
            nc.sync.dma_start(out=wd[:], in_=wkt[:])
            wsb = pa.tile([1, 512], dt.float32, name=f"wsb{li}", tag=f"wsb{li}")
            nc.sync.dma_start(out=wsb[:], in_=wd[:])
            wkts[li] = wsb

        def build_wrow(li, name):
            """wrow [1, 1024] col (u, h, n, kk) from the p-major [1, 512]
            bounce (identity layout -> one strided DVE copy per h)."""
            wsb = wkts[li]
            wrow = pa.tile([1, 1024], dt.float32, name=name, tag="wrow", bufs=1)
            wv = wrow[:].rearrange("o (u h q) -> o u h q", u=BL, h=2)
            sv = wsb[:].rearrange("o (u q) -> o u q", u=BL)
            for h in range(2):
                nc.vector.tensor_copy(out=wv[:, :, h, :], in_=sv[:])
            return wrow

        def broadcast_weights(wrow, name, out_dt):
            wb = pool.tile([128, 1024], out_dt, name=f"wb{name}")
            for c0 in (0, 512):
                wb_ps = ppsum.tile(
                    [128, 512], dt.float32, name=f"wbps{name}_{c0}", tag="wbps",
                    bufs=2,
                )
                nc.tensor.matmul(
                    wb_ps[:], ones1[:], wrow[:, c0:c0 + 512], start=True, stop=True,
                )
                nc.vector.tensor_copy(out=wb[:, c0:c0 + 512], in_=wb_ps[:])
            return wb

        wbs = {}
        for li in (2, 1, 0):
            wrow = build_wrow(li, f"wrow{li}")
            wbs[li] = broadcast_weights(
                wrow, f"L{li}", dt.bfloat16 if li else dt.float32,
            )
        stream_tile(0, 3, 2, nc.scalar)

        # ---- V tiles: col = u*64 + h*32 + n ----
        V = [pool.tile([128, 256], dt.float32, name=f"V{li}") for li in range(3)]

        def colsum(prod, name):
            ps = ppsum.tile([1, 256], dt.float32, name=f"ps{name}", tag="ps", bufs=2)
            nc.tensor.matmul(ps[:], ones[:], prod[:], start=True, stop=True)
            sb = pool.tile([1, 256], dt.float32, name=f"sb{name}")
            nc.vector.tensor_copy(out=sb[:], in_=ps[:])
            sbv = sb[:].rearrange("o (u h n) -> o u h n", u=BL, h=2)
            r = pool.tile([1, 128], dt.float32, name=f"r{name}")
            rv = r[:].rearrange("o (u n) -> o u n", u=BL)
            nc.vector.tensor_tensor(
                out=rv[:], in0=sbv[:, :, 0, :], in1=sbv[:, :, 1, :], op=AL.add,
            )
            return r

        results = {}
        done = set()

        def level_products(li):
            done.add(li)
            prod = pwork.tile([128, 256], dt.float32, name=f"pss{li}", tag="pc")
            nc.vector.tensor_tensor(
                out=prod[:], in0=V[li][:], in1=V[li][:], op=AL.mult,
            )
            results[f"ss{li}"] = colsum(prod, f"ss{li}")
            for (i, j) in PAIRS:
                if li in (i, j) and i in done and j in done:
                    prod = pwork.tile(
                        [128, 256], dt.float32, name=f"pd{i}{j}", tag="pc",
                    )
                    nc.vector.tensor_tensor(
                        out=prod[:], in0=V[i][:], in1=V[j][:], op=AL.mult,
                    )
                    results[f"d{i}{j}"] = colsum(prod, f"d{i}{j}")

        def gather_packed(li):
            HW = LEVELS[li][0] * LEVELS[li][1]
            og = pwork.tile([128, 512], dt.int32, name=f"ogp{li}", tag="ogp")
            nc.gpsimd.ap_gather(
                out_ap=og[:], in_ap=packed[li][:], idxs_ap=widxs[li][:],
                channels=128, num_elems=8 * HW, d=1, num_idxs=512,
            )
            prod = pwork.tile([128, 1024], dt.float32, name=f"lp{li}", tag="lp")
            nc.vector.tensor_tensor(
                out=prod[:], in0=og[:].bitcast(dt.bfloat16), in1=wbs[li][:],
                op=AL.mult,
            )
            nc.vector.tensor_reduce(
                out=V[li][:],
                in_=prod[:].rearrange("c (n f) -> c n f", f=4),
                axis=mybir.AxisListType.X, op=AL.add,
            )

        def gather_l0_img(u):
            og = pwork.tile([128, 256], dt.float32, name=f"og0_{u}", tag="og")
            nc.gpsimd.ap_gather(
                out_ap=og[:], in_ap=T_tiles[(0, u)][:],
                idxs_ap=widxs[0][:, u * 16:(u + 1) * 16],
                channels=128, num_elems=2 * HW0, d=1, num_idxs=256,
            )
            nc.vector.tensor_tensor(
                out=og[:], in0=og[:], in1=wbs[0][:, u * 256:(u + 1) * 256],
                op=AL.mult,
            )
            nc.vector.tensor_reduce(
                out=V[0][:, u * 64:(u + 1) * 64],
                in_=og[:].rearrange("c (n f) -> c n f", f=4),
                axis=mybir.AxisListType.X, op=AL.add,
            )

        gather_packed(2)
        level_products(2)
        gather_packed(1)
        level_products(1)
        for u in range(BL):
            gather_l0_img(u)
        level_products(0)

        # ---- cosine epilogue on partition 0 ----
        rns = []
        for li in range(3):
            nrm = pool.tile([1, 128], dt.float32, name=f"nrm{li}")
            nc.scalar.sqrt(out=nrm[:], in_=results[f"ss{li}"][:])
            nc.vector.tensor_scalar_max(out=nrm[:], in0=nrm[:], scalar1=EPS)
            rn = pool.tile([1, 128], dt.float32, name=f"rn{li}")
            nc.vector.reciprocal(out=rn[:], in_=nrm[:])
            rns.append(rn)

        tot = pool.tile([1, 128], dt.float32)
        first = True
        for i, j in PAIRS:
            t = pool.tile([1, 128], dt.float32, name=f"t{i}{j}")
            nc.vector.tensor_tensor(
                out=t[:], in0=results[f"d{i}{j}"][:], in1=rns[i][:], op=AL.mult,
            )
            nc.vector.tensor_tensor(out=t[:], in0=t[:], in1=rns[j][:], op=AL.mult)
            if first:
                nc.vector.tensor_copy(out=tot[:], in_=t[:])
                first = False
            else:
                nc.vector.tensor_tensor(out=tot[:], in0=tot[:], in1=t[:], op=AL.add)

        res = pool.tile([1, 1], dt.float32)
        nc.vector.tensor_reduce(
            out=res[:], in_=tot[:], axis=mybir.AxisListType.X, op=AL.add
        )
        nc.sync.dma_start(out=out.ap(), in_=res[:])

    nc.compile()
    return nc


def _get_program():
    if "nc" not in _CACHE:
        _CACHE["nc"] = _build_program()
    return _CACHE["nc"]


def _run_device(feat0, feat1, feat2, boxes, **run_kwargs):
    """Shard inputs batch-wise over the 8 cores, run the SPMD program, and
    return the BassKernelResults (one {"out": [1,1]} per core)."""
    from concourse.bass_utils import run_bass_kernel_spmd

    nc = _get_program()

    feats = [
        np.ascontiguousarray(np.asarray(f, dtype=np.float32))
        for f in (feat0, feat1, feat2)
    ]
    boxes = np.ascontiguousarray(np.asarray(boxes, dtype=np.float32))

    in_maps = []
    for k in range(N_CORES):
        sl = slice(k * BL, (k + 1) * BL)
        in_maps.append(
            {
                "feat0": feats[0][sl],
                "feat1": feats[1][sl],
                "feat2": feats[2][sl],
                "boxes": boxes[sl],
            }
        )

    return run_bass_kernel_spmd(
        nc, in_maps, core_ids=list(range(N_CORES)), **run_kwargs
    )


def kernel(feat0, feat1, feat2, boxes):
    r = _run_device(feat0, feat1, feat2, boxes)
    total = np.float64(0.0)
    for m in r.results:
        total += np.float64(m["out"].reshape(-1)[0])

    count = B * N * len(PAIRS)
    avg = np.float32(total) / np.float32(count)
    loss = np.float32(1.0) - avg
    loss = np.nan_to_num(loss, nan=0.0, posinf=1.0, neginf=0.0)
    return np.array(np.clip(loss, 0.0, 2.0), dtype=np.float32)


# revision 21
# speedup vs baseline: 1.1947x; 1.0217x over previous
"""Trainium2 Bass kernel for nn_CSCLoss: multi-scale bilinear point-sampling
cosine-consistency loss.

loss = 1 - mean_{pairs,(b,n)} <normalize(sample(feat_i, p_bn)), normalize(sample(feat_j, p_bn))>

Sharding: data-parallel over batch — 32 images -> 8 cores x 4 images; the
host sums the 8 per-core partial sums and applies the 1 - total/count
epilogue (the all-reduce of the sharding hint, done on 8 scalars).

Per-core dataflow (DMA- and gather-balanced, ~ridge):
 - Feature maps stream through SBUF as merged-channel-half tiles
   [128ch, 2*H*W]; L0 (64x64, 16 MiB) on the ACT HWDGE ring, L1+L2 on SP.
 - All per-point math runs point-per-partition ([128, k] tiles, boxes are
   DMA'd to a [128pt, 4] layout), so phase A is ~130 short DVE ops instead
   of serial single-lane [1, N] chains.
 - gpsimd.ap_gather extracts bilinear corners. Its cost is per-INDEX
   (~28 ns/idx + ~4 us/dispatch), so the index count is the knob:
    * L0: plain f32 gather, 4 corner idxs per point per ch-half, one
      256-idx dispatch per image (hides under L0's own DMA time).
    * L1/L2: tiles are re-packed into bf16 PAIR arrays (u32 word p holds
      bf16 pixels (p, p+1); even pairs are the bf16 cast itself, odd pairs
      are two strided u16 copies on the ACT engine). One u32 gather then
      yields BOTH x-corners -> 2 idxs per point per ch-half, one 512-idx
      dispatch per level.
 - Gather indices are computed directly as int16 [128pt, cols] values and
   shuffled into ap_gather's wrapped [16]-replicated layout by a tiny
   DRAM round-trip on the idle SP ring (read-back AP does the permutation
   and the x8 core replication via a 0-stride broadcast dim).
 - Corner lerp weights take the same route ([128pt, 12] -> DRAM ->
   [1, 1024] rows) and are broadcast to 128 partitions by a PE rank-1
   matmul; DVE applies them and reduces corner groups of 4 into sampled
   vectors V[c, u*64 + half*32 + n].
 - Channel reductions (squared norms, pairwise dots) are ones-vector
   matmuls accumulating [1, 256] in PSUM + a strided add folding the two
   channel halves; the cosine epilogue runs on partition 0 and emits one
   [1,1] partial per core.
"""

import sys
from contextlib import ExitStack

import numpy as np

if "/opt/trn_rl_repo" not in sys.path:
    sys.path.insert(0, "/opt/trn_rl_repo")

B, N, C = 32, 32, 256
LEVELS = [(64, 64), (32, 32), (16, 16)]  # (H, W)
N_CORES = 8
BL = B // N_CORES          # images per core
NPTS = BL * N              # 128 points per core
PAIRS = [(0, 1), (0, 2), (1, 2)]
EPS = 1e-12

_CACHE = {}


def _build_program():
    from concourse import bacc, bass, mybir, tile, library_config

    dt = mybir.dt
    AL = mybir.AluOpType

    nc = bacc.Bacc("TRN2", target_bir_lowering=False, debug=False)

    feats = [
        nc.dram_tensor(f"feat{i}", [BL, C, H, W], dt.float32, kind="ExternalInput")
        for i, (H, W) in enumerate(LEVELS)
    ]
    boxes = nc.dram_tensor("boxes", [BL, N, 4], dt.float32, kind="ExternalInput")
    out = nc.dram_tensor("out", [1, 1], dt.float32, kind="ExternalOutput")

    with tile.TileContext(nc) as tc, ExitStack() as ctx:
        pool = ctx.enter_context(tc.tile_pool(name="sbuf", bufs=1))
        pa = ctx.enter_context(tc.tile_pool(name="pa", bufs=1))
        pstream = ctx.enter_context(tc.tile_pool(name="stream", bufs=1))
        pwork = ctx.enter_context(tc.tile_pool(name="work", bufs=2))
        ppsum = ctx.enter_context(tc.tile_pool(name="psum", bufs=1, space="PSUM"))
        pdram = ctx.enter_context(tc.tile_pool(name="dram", bufs=1, space="DRAM"))

        nc.gpsimd.load_library(library_config.ap_gather)

        # ---- constants (DVE only — keep gpsimd free for gathers) ----
        ones1 = pool.tile([1, 128], dt.float32)
        nc.vector.memset(ones1[:], 1.0)
        ones = pool.tile([128, 1], dt.float32)
        nc.vector.memset(ones[:], 1.0)
        # per-partition u*2 (identity layout: p = pt = u*32 + n)
        u2f = pool.tile([128, 1], dt.float32)
        for u in range(BL):
            nc.vector.memset(u2f[u * N:(u + 1) * N, :], float(u * 2))

        # ---- boxes -> [128pt, 4] on the SP ring ----
        bxp = pool.tile([128, 4], dt.float32)
        nc.sync.dma_start(out=bxp[:], in_=boxes.rearrange("b n c -> (b n) c"))

        # ---- feature streaming DMAs ----
        # Queue plan (all queues are in-order; nothing early may wait on
        # anything late):  SP: boxes, T2 x4 (bufs=4, no reuse waits), idx
        # bounces, weight bounces, out.  ACT: T0 u0/u1, T1 x4 (bufs=4),
        # T0 u2, odd-pair copies, T0 u3 (its buffer-reuse wait on the first
        # L0 gather must not block the odd copies).
        fviews = [
            feats[li].rearrange("b (h c) hh ww -> c b h (hh ww)", h=2)
            for li in range(3)
        ]
        T_tiles = {}

        def stream_tile(li, u, bufs, eng):
            HW = LEVELS[li][0] * LEVELS[li][1]
            T = pstream.tile(
                [128, 2 * HW], dt.float32, name=f"T{li}_{u}",
                tag=f"T{li}", bufs=bufs,
            )
            eng.dma_start(
                out=T[:].rearrange("c (h q) -> c h q", h=2),
                in_=fviews[li][:, u],
            )
            T_tiles[(li, u)] = T

        HW0 = LEVELS[0][0] * LEVELS[0][1]
        for u in range(BL):
            stream_tile(2, u, 4, nc.sync)
        stream_tile(0, 0, 2, nc.scalar)
        stream_tile(0, 1, 2, nc.scalar)
        for u in range(BL):
            stream_tile(1, u, 4, nc.scalar)

        # ---- phase A ([128pt, k] math) ----
        def axis_prep(coord, E, name):
            """pf = clip(c*(E-1), 0, E-1); e0 = clamp(floor(pf), 0, E-2);
            we = pf - e0.  floor via 16.16 fixed point. All [128, 1]."""
            pf = pa.tile([128, 1], dt.float32, name=f"pf{name}", tag=f"pf{name}")
            nc.vector.tensor_scalar(
                out=pf[:], in0=coord, scalar1=float(E - 1), scalar2=0.0,
                op0=AL.mult, op1=AL.max,
            )
            nc.vector.tensor_scalar_min(out=pf[:], in0=pf[:], scalar1=float(E - 1))
            pxs = pa.tile([128, 1], dt.float32, name=f"pxs{name}", tag="pxs")
            nc.vector.tensor_scalar(
                out=pxs[:], in0=pf[:], scalar1=65536.0, scalar2=None, op0=AL.mult,
            )
            ifx = pa.tile([128, 1], dt.int32, name=f"ifx{name}", tag="ifx")
            nc.vector.tensor_copy(out=ifx[:], in_=pxs[:])
            x0i = pa.tile([128, 1], dt.int32, name=f"x0i{name}", tag="x0i")
            nc.vector.tensor_scalar(
                out=x0i[:], in0=ifx[:], scalar1=16, scalar2=None,
                op0=AL.arith_shift_right,
            )
            e0 = pa.tile([128, 1], dt.float32, name=f"e0{name}", tag=f"e0{name}")
            nc.vector.tensor_copy(out=e0[:], in_=x0i[:])
            nc.vector.tensor_scalar_min(out=e0[:], in0=e0[:], scalar1=float(E - 2))
            we = pa.tile([128, 1], dt.float32, name=f"we{name}", tag=f"we{name}")
            nc.vector.tensor_tensor(out=we[:], in0=pf[:], in1=e0[:], op=AL.subtract)
            return e0, we

        sdram = {}       # li -> DRAM bounce tile for indices
        s16_tiles = {}   # li -> int16 [128, cols] idx values
        wefs = {}        # li -> (wx, wy)

        def prep_level(li):
            H, W = LEVELS[li]
            HW = H * W
            x0f, wx = axis_prep(bxp[:, 0:1], W, f"x{li}")
            y0f, wy = axis_prep(bxp[:, 1:2], H, f"y{li}")
            wefs[li] = (wx, wy)
            qf = pa.tile([128, 1], dt.float32, name=f"qf{li}", tag="qf")
            nc.vector.tensor_scalar(
                out=qf[:], in0=y0f[:], scalar1=float(W), scalar2=None, op0=AL.mult,
            )
            nc.vector.tensor_tensor(out=qf[:], in0=qf[:], in1=x0f[:], op=AL.add)
            if li == 0:
                # S [128, 8] cols (h, k): q + h*HW + dk
                S = pa.tile([128, 8], dt.float32, name="S0f", tag="S0f")
                DK = [0.0, 1.0, float(W), float(W + 1)]
                for h in range(2):
                    for k in range(4):
                        nc.vector.tensor_scalar(
                            out=S[:, h * 4 + k:h * 4 + k + 1], in0=qf[:],
                            scalar1=DK[k] + h * float(HW), scalar2=None, op0=AL.add,
                        )
                S16 = pa.tile([128, 8], dt.int16, name="S0i", tag="S0i")
                nc.vector.tensor_copy(out=S16[:], in_=S[:])
                sd = pdram.tile([1, 1024], dt.int16, name="sd0")
            else:
                HW2 = HW // 2
                qi = pa.tile([128, 1], dt.int32, name=f"qi{li}", tag="qi")
                nc.vector.tensor_copy(out=qi[:], in_=qf[:])
                pari = pa.tile([128, 1], dt.int32, name=f"pari{li}", tag="pari")
                nc.vector.tensor_scalar(
                    out=pari[:], in0=qi[:], scalar1=1, scalar2=None,
                    op0=AL.bitwise_and,
                )
                shi = pa.tile([128, 1], dt.int32, name=f"shi{li}", tag="shi")
                nc.vector.tensor_scalar(
                    out=shi[:], in0=qi[:], scalar1=1, scalar2=None,
                    op0=AL.arith_shift_right,
                )
                parf = pa.tile([128, 1], dt.float32, name=f"parf{li}", tag="parf")
                nc.vector.tensor_copy(out=parf[:], in_=pari[:])
                shf = pa.tile([128, 1], dt.float32, name=f"shf{li}", tag="shf")
                nc.vector.tensor_copy(out=shf[:], in_=shi[:])
                # slot = (q>>1) + par*HW2; base = slot + u*2*HW
                slotf = pa.tile([128, 1], dt.float32, name=f"slotf{li}", tag="slotf")
                nc.vector.tensor_scalar(
                    out=slotf[:], in0=parf[:], scalar1=float(HW2), scalar2=None,
                    op0=AL.mult,
                )
                nc.vector.tensor_tensor(
                    out=slotf[:], in0=slotf[:], in1=shf[:], op=AL.add,
                )
                basef = pa.tile([128, 1], dt.float32, name=f"basef{li}", tag="basef")
                nc.vector.tensor_scalar(
                    out=basef[:], in0=u2f[:], scalar1=float(HW), scalar2=None,
                    op0=AL.mult,
                )
                nc.vector.tensor_tensor(
                    out=basef[:], in0=basef[:], in1=slotf[:], op=AL.add,
                )
                # S [128, 4] cols (h, row): base + h*HW + row*(W//2)
                S = pa.tile([128, 4], dt.float32, name=f"Sf{li}", tag=f"Sf{li}")
                for h in range(2):
                    for row in range(2):
                        nc.vector.tensor_scalar(
                            out=S[:, h * 2 + row:h * 2 + row + 1], in0=basef[:],
                            scalar1=float(h * HW + row * (W // 2)), scalar2=None,
                            op0=AL.add,
                        )
                S16 = pa.tile([128, 4], dt.int16, name=f"Si{li}", tag=f"Si{li}")
                nc.vector.tensor_copy(out=S16[:], in_=S[:])
                sd = pdram.tile([1, 512], dt.int16, name=f"sd{li}")
            sdram[li] = sd
            s16_tiles[li] = S16

        widxs = {}

        def wrap_idx(li):
            """S16 [128, ncol] -> DRAM p-major -> widx via x8 0-stride
            broadcast reads.  The p-major dump IS a valid wrapped layout
            for gather column j = ((p%%8)*ncol + col)*16 + p//8 (whole
            levels), resp. j = ((p'%%2)*8 + col)*16 + p'//2 per L0 image."""
            ncol = 8 if li == 0 else 4
            NIDX = 128 * ncol
            sdA = pdram.tile([1, NIDX], dt.int16, name=f"sdA{li}")
            nc.sync.dma_start(out=sdA[:], in_=s16_tiles[li][:])
            if li == 0:
                widx = pool.tile([128, 64], dt.int16, name="widx0")
                for u in range(BL):
                    nc.sync.dma_start(
                        out=widx[:, u * 16:(u + 1) * 16],
                        in_=sdA[:, u * 256:(u + 1) * 256].to_broadcast(
                            [8, 256]
                        ),
                    )
            else:
                widx = pool.tile([128, 32], dt.int16, name=f"widx{li}")
                nc.sync.dma_start(
                    out=widx[:], in_=sdA[:].to_broadcast([8, NIDX]),
                )
            widxs[li] = widx

        # ---- bf16 pair packing for L1/L2 (DVE casts + ACT odd copies) ----
        packed = {}

        def pack_level(li):
            HW = LEVELS[li][0] * LEVELS[li][1]
            HW2 = HW // 2
            P32 = pool.tile([128, 8 * HW], dt.int32, name=f"P32_{li}")
            Pb = P32[:].bitcast(dt.bfloat16)  # [128, 16*HW]
            for u in range(BL):
                T = T_tiles[(li, u)]
                for h in range(2):
                    base = (u * 2 + h) * 2 * HW
                    nc.vector.tensor_copy(
                        out=Pb[:, base:base + HW],
                        in_=T[:, h * HW:(h + 1) * HW],
                    )
                    bview = Pb[:, base:base + HW].rearrange(
                        "c (p two) -> c p two", two=2,
                    )
                    oview = Pb[:, base + HW:base + 2 * HW].rearrange(
                        "c (p two) -> c p two", two=2,
                    )
                    nc.scalar.copy(out=oview[:, :, 0], in_=bview[:, :, 1])
                    nc.scalar.copy(
                        out=oview[:, 0:HW2 - 1, 1], in_=bview[:, 1:HW2, 0],
                    )
                    # last odd slot's 2nd element is never indexed; zero it
                    # so the tile is fully initialized
                    nc.vector.memset(oview[:, HW2 - 1:HW2, 1], 0.0)
            packed[li] = P32

        # per level: phase A -> index wrap -> packing (L2 first: its gather
        # can start the moment the gpsimd library load finishes)
        prep_level(2)
        wrap_idx(2)
        pack_level(2)
        prep_level(1)
        wrap_idx(1)
        pack_level(1)
        stream_tile(0, 2, 2, nc.scalar)
        prep_level(0)
        wrap_idx(0)

        # ---- corner weights: [128(pt), 4] -> DRAM -> [1,512] -> wrow ----
        wkts = {}
        for li in range(3):
            wx, wy = wefs[li]
            w1x = pa.tile([128, 1], dt.float32, name=f"w1x{li}", tag="w1x")
            nc.vector.tensor_scalar(
                out=w1x[:], in0=wx[:], scalar1=-1.0, scalar2=1.0,
                op0=AL.mult, op1=AL.add,
            )
            w1y = pa.tile([128, 1], dt.float32, name=f"w1y{li}", tag="w1y")
            nc.vector.tensor_scalar(
                out=w1y[:], in0=wy[:], scalar1=-1.0, scalar2=1.0,
                op0=AL.mult, op1=AL.add,
            )
            wkt = pa.tile([128, 4], dt.float32, name=f"wkt{li}", tag=f"wkt{li}")
            for k, (wyt, wxt) in enumerate(
                [(w1y, w1x), (w1y, wx), (wy, w1x), (wy, wx)]
            ):
                nc.vector.tensor_tensor(
                    out=wkt[:, k:k + 1], in0=wyt[:], in1=wxt[:], op=AL.mult,
                )
            wd = pdram.tile([1, 512], dt.float32, name=f"wd{li}")
            nc.sync.dma_start(out=wd[:], in_=wkt[:])
            wsb = pa.tile([1, 512], dt.float32, name=f"wsb{li}", tag=f"wsb{li}")
            nc.sync.dma_start(out=wsb[:], in_=wd[:])
            wkts[li] = wsb

        def build_wrow(li, name):
            """wrow [1, 1024] in the relabeled og column order, assembled
            from the p-major [1, 512] bounce by strided DVE copies."""
            wsb = wkts[li]
            wrow = pa.tile([1, 1024], dt.float32, name=name, tag="wrow", bufs=1)
            if li == 0:
                # og col (img u) = pm*128 + h*64 + k*16 + pd; wrow col =
                # u*256 + that; src flat = (u*32 + pd*2 + pm)*4 + k
                wv = wrow[:].rearrange(
                    "o (u pm h k pd) -> o u pm h k pd", u=BL, pm=2, h=2, k=4,
                )
                sv = wsb[:].rearrange(
                    "o (u pd pm k) -> o u pd pm k", u=BL, pd=16, pm=2,
                )
                for u in range(BL):
                    for h in range(2):
                        nc.vector.tensor_copy(
                            out=wv[:, u, :, h, :, :],
                            in_=sv[:, u].transpose([0, 2, 3, 1]),
                        )
            else:
                # og col = pl*128 + h*64 + row*32 + ph*2 + xi;
                # src flat = (ph*8 + pl)*4 + row*2 + xi
                wv = wrow[:].rearrange(
                    "o (pl h row ph xi) -> o pl h row ph xi", pl=8, h=2, row=2,
                    ph=16,
                )
                sv = wsb[:].rearrange(
                    "o (ph pl row xi) -> o ph pl row xi", ph=16, pl=8, row=2,
                )
                for h in range(2):
                    nc.vector.tensor_copy(
                        out=wv[:, :, h, :, :, :],
                        in_=sv[:].transpose([0, 2, 3, 1, 4]),
                    )
            return wrow

        def broadcast_weights(wrow, name, out_dt):
            wb = pool.tile([128, 1024], out_dt, name=f"wb{name}")
            for c0 in (0, 512):
                wb_ps = ppsum.tile(
                    [128, 512], dt.float32, name=f"wbps{name}_{c0}", tag="wbps",
                    bufs=2,
                )
                nc.tensor.matmul(
                    wb_ps[:], ones1[:], wrow[:, c0:c0 + 512], start=True, stop=True,
                )
                nc.vector.tensor_copy(out=wb[:, c0:c0 + 512], in_=wb_ps[:])
            return wb

        wbs = {}
        for li in (2, 1, 0):
            wrow = build_wrow(li, f"wrow{li}")
            wbs[li] = broadcast_weights(
                wrow, f"L{li}", dt.bfloat16 if li else dt.float32,
            )
        stream_tile(0, 3, 2, nc.scalar)

        # ---- V tiles: col = u*64 + h*32 + n ----
        V = [pool.tile([128, 256], dt.float32, name=f"V{li}") for li in range(3)]

        def colsum(prod, name):
            ps = ppsum.tile([1, 256], dt.float32, name=f"ps{name}", tag="ps", bufs=2)
            nc.tensor.matmul(ps[:], ones[:], prod[:], start=True, stop=True)
            sb = pool.tile([1, 256], dt.float32, name=f"sb{name}")
            nc.vector.tensor_copy(out=sb[:], in_=ps[:])
            sbv = sb[:].rearrange("o (pl h ph) -> o pl h ph", pl=8, h=2)
            r = pool.tile([1, 128], dt.float32, name=f"r{name}")
            rv = r[:].rearrange("o (pl ph) -> o pl ph", pl=8)
            nc.vector.tensor_tensor(
                out=rv[:], in0=sbv[:, :, 0, :], in1=sbv[:, :, 1, :], op=AL.add,
            )
            return r

        results = {}
        done = set()

        def level_products(li):
            done.add(li)
            prod = pwork.tile([128, 256], dt.float32, name=f"pss{li}", tag="pc")
            nc.vector.tensor_tensor(
                out=prod[:], in0=V[li][:], in1=V[li][:], op=AL.mult,
            )
            results[f"ss{li}"] = colsum(prod, f"ss{li}")
            for (i, j) in PAIRS:
                if li in (i, j) and i in done and j in done:
                    prod = pwork.tile(
                        [128, 256], dt.float32, name=f"pd{i}{j}", tag="pc",
                    )
                    nc.vector.tensor_tensor(
                        out=prod[:], in0=V[i][:], in1=V[j][:], op=AL.mult,
                    )
                    results[f"d{i}{j}"] = colsum(prod, f"d{i}{j}")

        def gather_packed(li):
            HW = LEVELS[li][0] * LEVELS[li][1]
            og = pwork.tile([128, 512], dt.int32, name=f"ogp{li}", tag="ogp")
            nc.gpsimd.ap_gather(
                out_ap=og[:], in_ap=packed[li][:], idxs_ap=widxs[li][:],
                channels=128, num_elems=8 * HW, d=1, num_idxs=512,
            )
            # og col = pl*128 + h*64 + row*32 + ph*2 + xi
            prod = pwork.tile([128, 1024], dt.float32, name=f"lp{li}", tag="lp")
            nc.vector.tensor_tensor(
                out=prod[:], in0=og[:].bitcast(dt.bfloat16), in1=wbs[li][:],
                op=AL.mult,
            )
            t1 = pwork.tile([128, 512], dt.float32, name=f"t1{li}", tag="t1")
            nc.vector.tensor_reduce(
                out=t1[:],
                in_=prod[:].rearrange("c (a xi) -> c a xi", xi=2),
                axis=mybir.AxisListType.X, op=AL.add,
            )
            # t1 col = pl*64 + h*32 + row*16 + ph; reduce row -> V (pl, h, ph)
            nc.vector.tensor_reduce(
                out=V[li][:].rearrange("c (pl h ph) -> c pl h ph", pl=8, h=2),
                in_=t1[:].rearrange(
                    "c (pl h row ph) -> c pl h ph row", pl=8, h=2, row=2,
                ),
                axis=mybir.AxisListType.X, op=AL.add,
            )

        def gather_l0_img(u):
            og = pwork.tile([128, 256], dt.float32, name=f"og0_{u}", tag="og")
            nc.gpsimd.ap_gather(
                out_ap=og[:], in_ap=T_tiles[(0, u)][:],
                idxs_ap=widxs[0][:, u * 16:(u + 1) * 16],
                channels=128, num_elems=2 * HW0, d=1, num_idxs=256,
            )
            # og col = pm*128 + h*64 + k*16 + pd (p' = pd*2 + pm)
            nc.vector.tensor_tensor(
                out=og[:], in0=og[:], in1=wbs[0][:, u * 256:(u + 1) * 256],
                op=AL.mult,
            )
            tmpV = pwork.tile([128, 64], dt.float32, name=f"tv{u}", tag="tv")
            nc.vector.tensor_reduce(
                out=tmpV[:],
                in_=og[:].rearrange(
                    "c (pm h k pd) -> c pm h pd k", pm=2, h=2, k=4,
                ),
                axis=mybir.AxisListType.X, op=AL.add,
            )
            # scatter to V0 col = pdl*64 + pm*32 + h*16 + u*4 + pdh
            # (pt = u*32 + pd*2 + pm, pd = pdh*4 + pdl)
            nc.vector.tensor_copy(
                out=V[0][:].rearrange(
                    "c (pdl pm h u2 pdh) -> c u2 pm h pdh pdl", pdl=4, pm=2,
                    h=2, u2=BL,
                )[:, u],
                in_=tmpV[:].rearrange(
                    "c (pm h pdh pdl) -> c pm h pdh pdl", pm=2, h=2, pdh=4,
                ),
            )

        gather_packed(2)
        level_products(2)
        gather_packed(1)
        level_products(1)
        for u in range(BL):
            gather_l0_img(u)
        level_products(0)

        # ---- cosine epilogue on partition 0 ----
        rns = []
        for li in range(3):
            nrm = pool.tile([1, 128], dt.float32, name=f"nrm{li}")
            nc.scalar.sqrt(out=nrm[:], in_=results[f"ss{li}"][:])
            nc.vector.tensor_scalar_max(out=nrm[:], in0=nrm[:], scalar1=EPS)
            rn = pool.tile([1, 128], dt.float32, name=f"rn{li}")
            nc.vector.reciprocal(out=rn[:], in_=nrm[:])
            rns.append(rn)

        tot = pool.tile([1, 128], dt.float32)
        first = True
        for i, j in PAIRS:
            t = pool.tile([1, 128], dt.float32, name=f"t{i}{j}")
            nc.vector.tensor_tensor(
                out=t[:], in0=results[f"d{i}{j}"][:], in1=rns[i][:], op=AL.mult,
            )
            nc.vector.tensor_tensor(out=t[:], in0=t[:], in1=rns[j][:], op=AL.mult)
            if first:
                nc.vector.tensor_copy(out=tot[:], in_=t[:])
                first = False
            else:
                nc.vector.tensor_tensor(out=tot[:], in0=tot[:], in1=t[:], op=AL.add)

        res = pool.tile([1, 1], dt.float32)
        nc.vector.tensor_reduce(
            out=res[:], in_=tot[:], axis=mybir.AxisListType.X, op=AL.add
        )
        nc.sync.dma_start(out=out.ap(), in_=res[:])

    nc.compile()
    return nc


def _get_program():
    if "nc" not in _CACHE:
        _CACHE["nc"] = _build_program()
    return _CACHE["nc"]


def _run_device(feat0, feat1, feat2, boxes, **run_kwargs):
    """Shard inputs batch-wise over the 8 cores, run the SPMD program, and
    return the BassKernelResults (one {"out": [1,1]} per core)."""
    from concourse.bass_utils import run_bass_kernel_spmd

    nc = _get_program()

    feats = [
        np.ascontiguousarray(np.asarray(f, dtype=np.float32))
        for f in (feat0, feat1, feat2)
    ]
    boxes = np.ascontiguousarray(np.asarray(boxes, dtype=np.float32))

    in_maps = []
    for k in range(N_CORES):
        sl = slice(k * BL, (k + 1) * BL)
        in_maps.append(
            {
                "feat0": feats[0][sl],
                "feat1": feats[1][sl],
                "feat2": feats[2][sl],
                "boxes": boxes[sl],
            }
        )

    return run_bass_kernel_spmd(
        nc, in_maps, core_ids=list(range(N_CORES)), **run_kwargs
    )


def kernel(feat0, feat1, feat2, boxes):
    r = _run_device(feat0, feat1, feat2, boxes)
    total = np.float64(0.0)
    for m in r.results:
        total += np.float64(m["out"].reshape(-1)[0])

    count = B * N * len(PAIRS)
    avg = np.float32(total) / np.float32(count)
    loss = np.float32(1.0) - avg
    loss = np.nan_to_num(loss, nan=0.0, posinf=1.0, neginf=0.0)
    return np.array(np.clip(loss, 0.0, 2.0), dtype=np.float32)
